# revision 1
# baseline (speedup 1.0000x reference)
"""KMeans tokenizer (VQ codebook argmin) Trainium2 kernel.

Strategy (data-parallel over tokens, 8 cores):
  - host: reshape patches [16,2048,64] -> [32768,64], shard 4096 tokens/core,
    transpose to [64,4096]; vocab transposed to [64,8192], replicated.
  - device: argmin_c ||x-c||^2 == argmax_c (2x.c - |c|^2).
    Augmented K=65 fp32 matmul computes s = 2x.c - c2 directly in PSUM
    (row 64 of the stationary operand is ones, row 64 of the moving vocab
    operand is -c2, computed on device via ACT Square + PE ones-reduction).
    ACT copies PSUM chunks to an SBUF score row [128, 8192] (fp32, exact);
    DVE max8 + max_index extract the argmax index per token (first-occurrence
    tie semantics, matching jnp.argmin).
"""
import numpy as np
from contextlib import ExitStack

import concourse.bass as bass
import concourse.bacc as bacc
import concourse.tile as tile
import concourse.mybir as mybir
from concourse.bass_utils import run_bass_kernel_spmd

dt = mybir.dt

NCORES = 8
D = 64          # vector dim
KAUG = 65       # augmented contraction dim
NV = 8192       # vocab size
TPC = 4096      # tokens per core
NTILE = TPC // 128   # 32 token tiles
NCHUNK = NV // 512   # 16 code chunks

_NC_CACHE = {}


def _build():
    nc = bacc.Bacc("TRN2", target_bir_lowering=False, debug=False)
    pT = nc.dram_tensor("pT", [D, TPC], dt.float32, kind="ExternalInput")
    vT = nc.dram_tensor("vT", [D, NV], dt.float32, kind="ExternalInput")
    out = nc.dram_tensor("out", [128, NTILE], dt.int32, kind="ExternalOutput")

    with tile.TileContext(nc) as tc, ExitStack() as ctx:
        sb = ctx.enter_context(tc.tile_pool(name="sb", bufs=1))
        srow_pool = ctx.enter_context(tc.tile_pool(name="srow", bufs=2))
        small = ctx.enter_context(tc.tile_pool(name="small", bufs=2))
        ps = ctx.enter_context(
            tc.tile_pool(name="ps", bufs=4, space=bass.MemorySpace.PSUM)
        )
        ps1 = ctx.enter_context(
            tc.tile_pool(name="ps1", bufs=2, space=bass.MemorySpace.PSUM)
        )

        vt_in = sb.tile([D, NV], dt.float32, tag="vt_in")
        pt_in = sb.tile([D, TPC], dt.float32, tag="pt_in")
        nc.sync.dma_start(vt_in[:], vT.ap())
        nc.sync.dma_start(pt_in[:], pT.ap())

        # augmented moving operand: rows 0..63 = 2*vocabT, row 64 = -c2
        avT = sb.tile([KAUG, NV], dt.float32, tag="avT")
        nc.scalar.mul(avT[0:D, :], vt_in[:], 2.0)

        # c2 = sum_d vocabT[d,c]^2 via ACT Square + PE ones-matmul
        sq = sb.tile([D, NV], dt.float32, tag="sq")
        nc.scalar.activation(sq[:], vt_in[:], mybir.ActivationFunctionType.Square)
        ones = sb.tile([D, 1], dt.float32, tag="ones")
        nc.vector.memset(ones[:], 1.0)
        for ch in range(NCHUNK):
            c2p = ps1.tile([1, 512], dt.float32, tag="c2p")
            nc.tensor.matmul(
                c2p[:], ones[:], sq[:, bass.ts(ch, 512)], start=True, stop=True
            )
            # row 64 of avT gets -c2
            nc.vector.tensor_scalar_mul(
                avT[D : D + 1, bass.ts(ch, 512)], c2p[:], -1.0
            )

        # augmented stationary operand: rows 0..63 = patchesT, row 64 = ones
        apT = sb.tile([KAUG, TPC], dt.float32, tag="apT")
        nc.scalar.copy(apT[0:D, :], pt_in[:])
        nc.vector.memset(apT[D : D + 1, :], 1.0)

        outbuf = sb.tile([128, NTILE], dt.int32, tag="outbuf")

        for t in range(NTILE):
            srow = srow_pool.tile([128, NV], dt.float32, tag="srow")
            for ch in range(NCHUNK):
                p = ps.tile([128, 512], dt.float32, tag="s")
                nc.tensor.matmul(
                    p[:],
                    apT[:, bass.ts(t, 128)],
                    avT[:, bass.ts(ch, 512)],
                    start=True,
                    stop=True,
                )
                nc.scalar.copy(srow[:, bass.ts(ch, 512)], p[:])
            top8 = small.tile([128, 8], dt.float32, tag="top8")
            idx8 = small.tile([128, 8], dt.uint32, tag="idx8")
            nc.vector.max(top8[:], srow[:])
            nc.vector.max_index(idx8[:], top8[:], srow[:])
            nc.vector.tensor_copy(outbuf[:, t : t + 1], idx8[:, 0:1])

        nc.sync.dma_start(out.ap(), outbuf[:])

    nc.compile()
    return nc


def kernel(patches: np.ndarray, vocab: np.ndarray) -> np.ndarray:
    patches = np.asarray(patches, dtype=np.float32)
    vocab = np.asarray(vocab, dtype=np.float32)
    B, N, _ = patches.shape  # (16, 2048, 64)

    if "nc" not in _NC_CACHE:
        _NC_CACHE["nc"] = _build()
    nc = _NC_CACHE["nc"]

    flat = patches.reshape(-1, D)  # [32768, 64]
    vTn = np.ascontiguousarray(vocab.T)  # [64, 8192]
    in_maps = []
    for c in range(NCORES):
        shard = flat[c * TPC : (c + 1) * TPC]  # [4096, 64]
        in_maps.append(
            {"pT": np.ascontiguousarray(shard.T), "vT": vTn}
        )

    res = run_bass_kernel_spmd(nc, in_maps, core_ids=list(range(NCORES)))

    parts = []
    for c in range(NCORES):
        arr = res.results[c]["out"]  # [128, 32] int32, token g = t*128 + p
        parts.append(np.ascontiguousarray(arr.T).reshape(-1))
    tokens = np.concatenate(parts).reshape(B, N).astype(np.int32)
    return tokens
